# revision 34
# baseline (speedup 1.0000x reference)
"""CEM sampling kernel for Trainium2, 8-core SPMD (population sharded).

V2: the DTW min-plus DP runs in f16 on DVE (TT-min gets the 2x 16-bit
DVE mode: 194ns vs 328ns at width 257; the scan's carry chain is
internally fp32 and costs 328ns either way, so only one f16 rounding
per cell). obs chunks are DMA'd f32 into a staging arena and converted
f32->f16 by the otherwise-idle Pool engine (the +inf separator column
lives in the staging arena and converts to f16 inf each time).

Per core (512 of 4096 members) the DP is two independent packed
257-wide chains (two 128-member lanes each, inf separator), ops
interleaved per row so each chain's write-ack latency hides under the
other chain's ops (min-TT and the scan are DVE-only opcodes on real
TRN2; neuronxcc rejects them on Pool/GpSimd).

The clip in actions = clip(means + stds*noise) is removed
algebraically: ACT computes z = relu(2 - relu(stds*n + means + 1))
(actions = 1 - z) into an f16 [T, A, PL] layout during the DP window;
statistics are rebuilt from U = sum(w), V = sum(w z), Q = sum(w z^2)
after one fused AllReduce: num1 = U - V, num2 = U - 2V + Q.
Top-K: f16 AllGather of dists, 0-stride broadcast, rank = #(d_j <
d_own) via f16 is_lt compares (4x DVE mode). Stats accumulation is
split DVE/ACT/Pool by env knobs; ACT accums use Relu (wz >= 0) to stay
in the loaded activation-table set.
"""

import os
import sys

for _p in ("/opt/trn_rl_repo", "/root/.axon_site/_ro/trn_rl_repo"):
    if _p not in sys.path:
        sys.path.insert(0, _p)

import numpy as np

import concourse.bass as bass
import concourse.bacc as bacc
import concourse.tile as tile
from concourse import mybir
from concourse import bass_utils

F32 = mybir.dt.float32
F16 = mybir.dt.float16
ALU = mybir.AluOpType
ACTF = mybir.ActivationFunctionType

P, T, A = 4096, 128, 32
NCORES = 8
PL = P // NCORES          # 512 population per core
K = int(P * 0.1)          # 409
TEMP, MOM, MIN_STD = 0.5, 0.1, 0.05
INF = 1.0e30

RCH = 8
CHUNK_ROWS = [4, 4] + [RCH] * ((T - 8) // RCH)
NBS = int(os.environ.get("CEM_NBS", "2"))    # f32 staging bufs per pair
NBF = int(os.environ.get("CEM_NBF", "3"))    # f16 arena bufs per pair
W2 = 2 * T + 1            # 257 packed pair width (128 | sep | 128)
NOISE_C0 = int(os.environ.get("CEM_NOISE_C0", "1"))  # first group with a noise slice
NPOOL1 = int(os.environ.get("CEM_NPOOL1", "5"))  # blk1 pool-TT cols
NPOOL2 = int(os.environ.get("CEM_NPOOL2", "4"))  # blk2 pool-TT cols
NACT1 = int(os.environ.get("CEM_NACT1", "7"))    # blk1 ACT-accum cols
NACT2 = int(os.environ.get("CEM_NACT2", "6"))    # blk2 ACT-accum cols
POOLACC = int(os.environ.get("CEM_POOLACC", "0"))  # pool does its own accums
# exact periodic state re-base: subtract SHIFT from the whole DP state after
# each full 8-row group (uniform shift of a min-plus row state is exact; all
# members shift equally so ranks and exp-differences are unchanged). Keeps
# |state| ~ O(2) so the per-row f16 rounding stays ~1e-3 instead of ~2e-2.
SHIFT = float(os.environ.get("CEM_SHIFT", "2.337"))
WIDETT = int(os.environ.get("CEM_WIDETT", "0"))  # one TT spans both chains
GROUP = [list(range(NCORES))]

_CACHE = {}


def _build(stage=9, single=False):
    nc = bacc.Bacc(
        "TRN2",
        target_bir_lowering=False,
        debug=False,
        num_devices=1 if single else NCORES,
    )
    obs_d = nc.dram_tensor("obs", [PL, T, T], F32, kind="ExternalInput")
    means_d = nc.dram_tensor("means", [T, 1, A], F32, kind="ExternalInput")
    stds_d = nc.dram_tensor("stds", [T, 1, A], F32, kind="ExternalInput")
    noise_d = nc.dram_tensor("noise", [T, PL, A], F32, kind="ExternalInput")
    out_d = nc.dram_tensor("out", [2, T, 1, A], F32, kind="ExternalOutput")

    with tile.TileContext(nc) as tc:
        with (
            tc.tile_pool(name="main", bufs=1) as mp,
            tc.tile_pool(name="dram", bufs=1, space="DRAM") as dp,
        ):
            # ---------------- tiles
            means_t = mp.tile([T, A], F32)
            stds_t = mp.tile([T, A], F32)
            bias1_t = mp.tile([T, A], F32)       # means + 1
            two_t = mp.tile([T, 1], F32)
            noise_t = mp.tile([T, PL // 2, A], F32)
            z_t = mp.tile([T, A, PL], F16)       # z = relu(2 - relu(s*n+m+1))
            y_ring = mp.tile([T, 2, PL], F16)

            sar01 = mp.tile([128, NBS, RCH, W2], F32)  # f32 staging, sep=INF
            sar23 = mp.tile([128, NBS, RCH, W2], F32)
            far01 = mp.tile([128, NBF, RCH, W2], F16)  # f16 DP arenas
            far23 = mp.tile([128, NBF, RCH, W2], F16)
            pball = mp.tile([128, 2, W2 + 1], F16)
            uball = mp.tile([128, 2 * W2 + 1], F16)
            pbuf_d = pball[:, 0]
            pbuf_p = pball[:, 1]
            pbflat = pball.rearrange("p a w -> p (a w)")
            ubuf_d = uball[:, 0:W2]
            ubuf_p = uball[:, W2 + 1 : 2 * W2 + 1]

            down16 = mp.tile([128, 4], F16)
            down32 = mp.tile([128, 4], F32)
            gdb = mp.tile([128, 2, P // 2], F16)
            srank = mp.tile([128, P // 2], F16)
            rank8 = mp.tile([128, 2, 4], F32)
            rank4 = mp.tile([128, 4], F32)
            mask4 = mp.tile([128, 4], F32)
            gsb = mp.tile([128, P // 128], F16)
            pmin = mp.tile([128, 1], F16)
            pmb = mp.tile([128, 128], F16)
            ming = mp.tile([128, 1], F16)
            biast = mp.tile([128, 1], F32)
            e4 = mp.tile([128, 4], F32)
            w4 = mp.tile([128, 4], F16)
            epre = mp.tile([128, 1], F32)
            wrow = mp.tile([128, PL], F16)

            wz_d = mp.tile([128, 8, PL], F16)   # DVE-fed ring (2 slabs of 4)
            wzz_d = mp.tile([128, 8, PL], F16)
            wz_p = mp.tile([128, 6, PL], F16)   # Pool-fed ring
            wzz_p = mp.tile([128, 6, PL], F16)
            dump = mp.tile([128, 4, PL], F16)   # ACT-accum dump ring
            dmp2 = mp.tile([128, 4, PL], F16)   # DVE-accum dump ring
            vqu1 = mp.tile([128, 64], F32)      # blk1: a<16: V=2a,Q=2a+1; U=32
            vqu2 = mp.tile([128, 64], F32)      # blk2: a>=16
            rvqu1 = mp.tile([128, 64], F32)
            rvqu2 = mp.tile([128, 64], F32)

            ld16 = dp.tile([PL], F16)
            wl16 = dp.tile([PL], F16)
            gd16 = dp.tile([P], F16)
            pd16 = dp.tile([128], F16)
            arin1 = dp.tile([128 * 64], F32)
            arout1 = dp.tile([128 * 64], F32)
            arin2 = dp.tile([128 * 64], F32)
            arout2 = dp.tile([128 * 64], F32)

            # ---------------- init (before chunk DMAs: arena dep direction)
            nc.vector.memset(sar01[:, :, :, T : T + 1], INF)
            nc.gpsimd.memset(sar23[:, :, :, T : T + 1], INF)
            nc.vector.memset(pbuf_d[:], INF)
            nc.vector.memset(pbuf_d[:, 0:1], 0.0)
            nc.vector.memset(pbuf_d[:, T + 1 : T + 2], 0.0)
            nc.vector.memset(pbuf_p[:], INF)
            nc.vector.memset(pbuf_p[:, 0:1], 0.0)
            nc.vector.memset(pbuf_p[:, T + 1 : T + 2], 0.0)
            nc.vector.memset(two_t[:], 2.0)
            nc.gpsimd.memset(vqu1[:], 0.0)
            nc.gpsimd.memset(vqu2[:], 0.0)

            # scalar-queue DMAs: means/stds then noise slices
            nc.scalar.dma_start(means_t[:], means_d[:, 0, :])
            nc.scalar.dma_start(stds_t[:], stds_d[:, 0, :])
            nc.vector.tensor_scalar(bias1_t[:], means_t[:], 1.0, None, op0=ALU.add)
            NSL = 8
            PSL = PL // NSL

            PH = PL // 2

            def emit_z(h):
                # z = relu(2 - relu(s*n + m + 1)) for population half h
                for a in range(A):
                    nc.scalar.activation(
                        y_ring[:, a % 2, 0:PH],
                        noise_t[:, :, a],
                        ACTF.Relu,
                        bias=bias1_t[:, a : a + 1],
                        scale=stds_t[:, a : a + 1],
                    )
                    nc.scalar.activation(
                        z_t[:, a, h * PH : (h + 1) * PH],
                        y_ring[:, a % 2, 0:PH],
                        ACTF.Relu,
                        bias=two_t[:, 0:1],
                        scale=-1.0,
                    )

            # ---------------- DP: per-group [obs DMAs; pool f32->f16; rows]
            row0 = [0]
            for c, rows in enumerate(CHUNK_ROWS):
                row0.append(row0[-1] + rows)
            NCH = len(CHUNK_ROWS)

            for g in range(NCH):
                rows, r0 = CHUNK_ROWS[g], row0[g]
                bs = g % NBS
                bf = g % NBF
                nc.sync.dma_start(
                    sar01[:, bs, 0:rows, 0:T], obs_d[0:128, r0 : r0 + rows, :]
                )
                nc.sync.dma_start(
                    sar01[:, bs, 0:rows, T + 1 : W2],
                    obs_d[128:256, r0 : r0 + rows, :],
                )
                nc.sync.dma_start(
                    sar23[:, bs, 0:rows, 0:T], obs_d[256:384, r0 : r0 + rows, :]
                )
                nc.sync.dma_start(
                    sar23[:, bs, 0:rows, T + 1 : W2],
                    obs_d[384:512, r0 : r0 + rows, :],
                )
                if NOISE_C0 <= g < NOISE_C0 + 4:
                    s = g - NOISE_C0
                    nc.sync.dma_start(
                        noise_t[:, s * PSL : (s + 1) * PSL, :],
                        noise_d[:, s * PSL : (s + 1) * PSL, :],
                    )
                if g == NOISE_C0 + 4:
                    emit_z(0)
                # f32 -> f16 (sep col converts to inf); one op per pair
                nc.gpsimd.tensor_copy(
                    far01[:, bf, 0:rows, :], sar01[:, bs, 0:rows, :]
                )
                nc.gpsimd.tensor_copy(
                    far23[:, bf, 0:rows, :], sar23[:, bs, 0:rows, :]
                )
                for rr in range(rows):
                    r = r0 + rr
                    if WIDETT:
                        nc.vector.tensor_tensor(
                            uball[:], pbflat[:, 0 : 2 * W2 + 1],
                            pbflat[:, 1 : 2 * W2 + 2], op=ALU.min,
                        )
                    else:
                        nc.vector.tensor_tensor(
                            ubuf_d, pbuf_d[:, 0:W2], pbuf_d[:, 1 : W2 + 1],
                            op=ALU.min,
                        )
                        nc.vector.tensor_tensor(
                            ubuf_p, pbuf_p[:, 0:W2], pbuf_p[:, 1 : W2 + 1],
                            op=ALU.min,
                        )
                    nc.vector.tensor_tensor_scan(
                        pbuf_d[:, 1 : W2 + 1], ubuf_d, far01[:, bf, rr, :],
                        INF, op0=ALU.min, op1=ALU.add,
                    )
                    nc.vector.tensor_tensor_scan(
                        pbuf_p[:, 1 : W2 + 1], ubuf_p, far23[:, bf, rr, :],
                        INF, op0=ALU.min, op1=ALU.add,
                    )
                    if r == 0:
                        nc.vector.memset(pbuf_d[:, 0:1], INF)
                        nc.vector.memset(pbuf_p[:, 0:1], INF)
                # re-base the DP state after every 8th row except the end
                # (uniform min-plus shift: exact, keeps the f16 state small)
                if (r0 + rows) % RCH == 0 and r0 + rows < T:
                    nc.vector.tensor_scalar(
                        pbflat[:], pbflat[:], -SHIFT, None, op0=ALU.add
                    )

            # ---------------- noise half 1 (ACT queue: after z(0) reads)
            if stage >= 1:
                for s_ in range(4, NSL):
                    nc.scalar.dma_start(
                        noise_t[:, (s_ % 4) * PSL : (s_ % 4 + 1) * PSL, :],
                        noise_d[:, s_ * PSL : (s_ + 1) * PSL, :],
                    )
                emit_z(1)
                # preload Exp+Sqrt tables while the DP finishes
                nc.scalar.activation(epre[:], two_t[:], ACTF.Exp, bias=0.0, scale=1.0)
                nc.scalar.sqrt(epre[:], two_t[:])

            # ---------------- dists -> allgather -> broadcast -> ranks
            nc.vector.tensor_copy(down16[:, 0:1], pbuf_d[:, T : T + 1])
            nc.vector.tensor_copy(down16[:, 1:2], pbuf_d[:, W2 : W2 + 1])
            nc.vector.tensor_copy(down16[:, 2:3], pbuf_p[:, T : T + 1])
            nc.vector.tensor_copy(down16[:, 3:4], pbuf_p[:, W2 : W2 + 1])
            nc.vector.tensor_copy(down32[:], down16[:])

            nc.sync.dma_start(ld16.rearrange("(k p) -> p k", p=128), down16[:])
            if single:
                # split the local AllGather stand-in so the first gdb
                # broadcast can start one hop earlier
                for hh in range(2):
                    _, gsrc4 = bass.broadcast_tensor_aps(
                        gd16[hh * P // 2 : (hh + 1) * P // 2].rearrange(
                            "(r f) -> r f", r=NCORES // 2
                        ),
                        ld16.rearrange("(o f) -> o f", o=1),
                    )
                    nc.sync.dma_start(
                        gd16[hh * P // 2 : (hh + 1) * P // 2].rearrange(
                            "(r f) -> r f", r=NCORES // 2
                        ),
                        gsrc4,
                    )
            else:
                nc.gpsimd.collective_compute(
                    "AllGather",
                    ALU.bypass,
                    replica_groups=GROUP,
                    ins=[ld16.opt()],
                    outs=[gd16.opt()],
                )

            if stage >= 3:
                PH2 = P // 2
                # global-min chain first in the queue (tiny, unblocks pmin so
                # the pd/pmb round-trip hides under the rank compares)
                for h in range(2):
                    _, gsrc = bass.broadcast_tensor_aps(
                        gdb[:, h],
                        gd16[h * PH2 : (h + 1) * PH2].rearrange(
                            "(o f) -> o f", o=1
                        ),
                    )
                    nc.sync.dma_start(gdb[:, h], gsrc)
                nc.sync.dma_start(
                    gsb[:], gd16.rearrange("(p q) -> p q", p=128)
                )
                nc.vector.tensor_reduce(
                    pmin[:], gsb[:], axis=mybir.AxisListType.X, op=ALU.min
                )
                nc.sync.dma_start(
                    pd16.rearrange("(p q) -> p q", q=1), pmin[:]
                )
                _, psrc = bass.broadcast_tensor_aps(
                    pmb[:], pd16.rearrange("(o f) -> o f", o=1)
                )
                nc.sync.dma_start(pmb[:], psrc)

                for h in range(2):
                    for k in range(4):
                        nc.vector.tensor_scalar(
                            srank[:],
                            gdb[:, h],
                            down32[:, k : k + 1],
                            None,
                            op0=ALU.is_lt,
                            op1=ALU.add,
                            accum_out=rank8[:, h, k : k + 1],
                        )
                nc.vector.tensor_reduce(
                    ming[:], pmb[:], axis=mybir.AxisListType.X, op=ALU.min
                )
                nc.vector.tensor_scalar(
                    biast[:], ming[:], TEMP, None, op0=ALU.mult
                )
                nc.vector.tensor_tensor(
                    rank4[:], rank8[:, 0], rank8[:, 1], op=ALU.add
                )
                nc.vector.tensor_scalar(
                    mask4[:], rank4[:], float(K), None, op0=ALU.is_lt
                )
                nc.scalar.activation(
                    e4[:], down16[:], ACTF.Exp, bias=biast[:, 0:1], scale=-TEMP
                )
                nc.vector.tensor_tensor(w4[:], e4[:], mask4[:], op=ALU.mult)

                nc.scalar.dma_start(wl16.rearrange("(k p) -> p k", p=128), w4[:])
                _, wsrc = bass.broadcast_tensor_aps(
                    wrow[:], wl16.rearrange("(o f) -> o f", o=1)
                )
                nc.scalar.dma_start(wrow[:], wsrc)

            if stage >= 5:
                _actc = [0]
                _dvec = [0]

                def adump():
                    _actc[0] += 1
                    return dump[:, _actc[0] % 4]

                def ddump():
                    _dvec[0] += 1
                    return dmp2[:, _dvec[0] % 4]

                U = vqu1[:, 32:33]
                nc.vector.tensor_scalar(
                    ddump(), wrow[:], 0.0, None, op0=ALU.add, op1=ALU.add,
                    accum_out=U,
                )

                def V(a):
                    blk = vqu1 if a < 16 else vqu2
                    return blk[:, 2 * (a % 16) : 2 * (a % 16) + 1]

                def Q(a):
                    blk = vqu1 if a < 16 else vqu2
                    return blk[:, 2 * (a % 16) + 1 : 2 * (a % 16) + 2]

                # per 16-col block: pool-TT cols (DVE accums), DVE-TT cols
                # with ACT accums, then DVE-full cols. Block 1 first so its
                # AllReduce overlaps block-2 stats.
                for blk in range(2):
                    a0 = blk * 16
                    npool_b = NPOOL1 if blk == 0 else NPOOL2
                    nact_b = NACT1 if blk == 0 else NACT2
                    cols_pool = list(range(a0, a0 + npool_b))
                    cols_act = list(range(a0 + npool_b, a0 + npool_b + nact_b))
                    cols_dve = list(range(a0 + npool_b + nact_b, a0 + 16))
                    # pool TTs issue first (they gate nothing on DVE)
                    for i, a in enumerate(cols_pool):
                        za = z_t[:, a, :]
                        s = (blk * NPOOL1 + i) % 6
                        nc.gpsimd.tensor_tensor(
                            wz_p[:, s], za, wrow[:], op=ALU.mult
                        )
                        nc.gpsimd.tensor_tensor(
                            wzz_p[:, s], wz_p[:, s], za, op=ALU.mult
                        )
                    # DVE work: ACT-accum cols first, then DVE-full cols,
                    # then the pool-col accums (pool results ready by then)
                    for i, a in enumerate(cols_act):
                        za = z_t[:, a, :]
                        wz, wzz = wz_d[:, a % 4], wzz_d[:, a % 4]
                        nc.vector.tensor_tensor(wz, za, wrow[:], op=ALU.mult)
                        nc.vector.tensor_tensor(wzz, wz, za, op=ALU.mult)
                        nc.scalar.activation(
                            adump(), wz, ACTF.Relu, accum_out=V(a)
                        )
                        nc.scalar.activation(
                            adump(), wzz, ACTF.Relu, accum_out=Q(a)
                        )
                    for a in cols_dve:
                        za = z_t[:, a, :]
                        wz = wz_d[:, a % 4]
                        nc.vector.tensor_tensor(wz, za, wrow[:], op=ALU.mult)
                        nc.vector.tensor_scalar(
                            ddump(), wz, 0.0, None, op0=ALU.add,
                            op1=ALU.add, accum_out=V(a),
                        )
                        nc.vector.tensor_tensor(wz, wz, za, op=ALU.mult)
                        nc.vector.tensor_scalar(
                            ddump(), wz, 0.0, None, op0=ALU.add,
                            op1=ALU.add, accum_out=Q(a),
                        )
                    for i, a in enumerate(cols_pool):
                        s = (blk * NPOOL1 + i) % 6
                        nc.vector.tensor_scalar(
                            ddump(), wz_p[:, s], 0.0, None, op0=ALU.add,
                            op1=ALU.add, accum_out=V(a),
                        )
                        nc.vector.tensor_scalar(
                            ddump(), wzz_p[:, s], 0.0, None, op0=ALU.add,
                            op1=ALU.add, accum_out=Q(a),
                        )
                    vin = vqu1 if blk == 0 else vqu2
                    ain = arin1 if blk == 0 else arin2
                    aout = arout1 if blk == 0 else arout2
                    rv = rvqu1 if blk == 0 else rvqu2
                    q_ar = nc.sync if blk == 0 else nc.scalar
                    q_ar.dma_start(
                        ain.rearrange("(p a) -> p a", a=64), vin[:]
                    )
                    if single:
                        q_ar.dma_start(aout[:], ain[:])
                    else:
                        nc.gpsimd.collective_compute(
                            "AllReduce", ALU.add, replica_groups=GROUP,
                            ins=[ain.opt()], outs=[aout.opt()],
                        )
                    q_ar.dma_start(
                        rv[:], aout.rearrange("(p a) -> p a", a=64)
                    )

                # ---------------- final statistics (block 1 math first)
                inv = mp.tile([128, 1], F32)
                mh = mp.tile([128, A], F32)
                mstd = mp.tile([128, 2, A], F32)
                q = mp.tile([128, A], F32)
                t2 = mp.tile([128, A], F32)
                msq = mp.tile([128, A], F32)
                var = mp.tile([128, A], F32)
                rU = rvqu1[:, 32:33]
                nc.vector.reciprocal(inv[:], rU)
                for blk in range(2):
                    rsrc = rvqu1 if blk == 0 else rvqu2
                    n0 = blk * 16
                    rV = rsrc[:, 0:32:2]
                    rQ = rsrc[:, 1:32:2]
                    mhB = mh[:, n0 : n0 + 16]
                    qB = q[:, n0 : n0 + 16]
                    t2B = t2[:, n0 : n0 + 16]
                    msqB = msq[:, n0 : n0 + 16]
                    varB = var[:, n0 : n0 + 16]
                    stdB = mstd[:, 1, n0 : n0 + 16]
                    # num1 = U - V ; mean_hat = num1 / U
                    nc.vector.tensor_scalar(
                        t2B, rV, rU[:, 0:1], -1.0, op0=ALU.subtract, op1=ALU.mult
                    )
                    nc.vector.tensor_scalar(mhB, t2B, inv[:, 0:1], None, op0=ALU.mult)
                    # num2 = U - 2V + Q ; q = num2 / U
                    nc.vector.tensor_scalar(t2B, rV, -2.0, None, op0=ALU.mult)
                    nc.vector.tensor_tensor(t2B, t2B, rQ, op=ALU.add)
                    nc.vector.tensor_scalar(t2B, t2B, rU[:, 0:1], None, op0=ALU.add)
                    nc.vector.tensor_scalar(qB, t2B, inv[:, 0:1], None, op0=ALU.mult)
                    nc.vector.tensor_tensor(msqB, mhB, mhB, op=ALU.mult)
                    nc.vector.tensor_tensor(varB, qB, msqB, op=ALU.subtract)
                    nc.vector.tensor_scalar(varB, varB, 0.0, None, op0=ALU.max)
                    nc.scalar.sqrt(stdB, varB)
                    nc.vector.tensor_scalar(
                        stdB, stdB, MIN_STD, 1.0, op0=ALU.max, op1=ALU.min
                    )
                    nc.vector.tensor_scalar(mhB, mhB, 1.0 - MOM, None, op0=ALU.mult)
                    nc.vector.scalar_tensor_tensor(
                        mstd[:, 0, n0 : n0 + 16], means_t[:, n0 : n0 + 16], MOM,
                        mhB, op0=ALU.mult, op1=ALU.add,
                    )
                nc.sync.dma_start(out_d[0, :, 0, :], mstd[:, 0, :])
                nc.scalar.dma_start(out_d[1, :, 0, :], mstd[:, 1, :])
            else:
                dbg = mp.tile([128, A], F32)
                nc.vector.memset(dbg[:], 0.0)
                if stage >= 3:
                    nc.vector.tensor_copy(dbg[:, 0:4], w4[:])
                    nc.vector.tensor_copy(dbg[:, 4:8], rank4[:])
                else:
                    nc.vector.tensor_copy(dbg[:, 0:4], down16[:])
                nc.scalar.dma_start(out_d[0, :, 0, :], dbg[:])
                nc.scalar.dma_start(out_d[1, :, 0, :], dbg[:])

    nc.compile()
    return nc


def _get_nc(stage=None, single=None):
    if stage is None:
        stage = int(os.environ.get("CEM_STAGE", "9"))
    if single is None:
        single = bool(int(os.environ.get("CEM_SINGLE", "0")))
    key = ("nc", stage, single)
    if key not in _CACHE:
        _CACHE[key] = _build(stage, single)
    return _CACHE[key]


def kernel(**inputs):
    obs = np.ascontiguousarray(np.asarray(inputs["obs_diffs"], np.float32))
    means = np.ascontiguousarray(np.asarray(inputs["means"], np.float32))
    stds = np.ascontiguousarray(np.asarray(inputs["stds"], np.float32))
    noise = np.ascontiguousarray(np.asarray(inputs["noise"], np.float32))

    nc = _get_nc(stage=9, single=False)
    in_maps = []
    for c in range(NCORES):
        in_maps.append(
            {
                "obs": obs[c * PL : (c + 1) * PL],
                "means": means,
                "stds": stds,
                "noise": np.ascontiguousarray(noise[:, c * PL : (c + 1) * PL, :]),
            }
        )
    res = bass_utils.run_bass_kernel_spmd(
        nc, in_maps, core_ids=list(range(NCORES))
    )
    out = np.asarray(res.results[0]["out"], np.float32)
    return out.reshape(2, T, 1, A)


# revision 35
# speedup vs baseline: 1.0265x; 1.0265x over previous
"""CEM sampling kernel for Trainium2, 8-core SPMD (population sharded).

V2: the DTW min-plus DP runs in f16 on DVE (TT-min gets the 2x 16-bit
DVE mode: 194ns vs 328ns at width 257; the scan's carry chain is
internally fp32 and costs 328ns either way, so only one f16 rounding
per cell). obs chunks are DMA'd f32 into a staging arena and converted
f32->f16 by the otherwise-idle Pool engine (the +inf separator column
lives in the staging arena and converts to f16 inf each time).

Per core (512 of 4096 members) the DP is two independent packed
257-wide chains (two 128-member lanes each, inf separator), ops
interleaved per row so each chain's write-ack latency hides under the
other chain's ops (min-TT and the scan are DVE-only opcodes on real
TRN2; neuronxcc rejects them on Pool/GpSimd).

The clip in actions = clip(means + stds*noise) is removed
algebraically: ACT computes z = relu(2 - relu(stds*n + means + 1))
(actions = 1 - z) into an f16 [T, A, PL] layout during the DP window;
statistics are rebuilt from U = sum(w), V = sum(w z), Q = sum(w z^2)
after one fused AllReduce: num1 = U - V, num2 = U - 2V + Q.
Top-K: f16 AllGather of dists, 0-stride broadcast, rank = #(d_j <
d_own) via f16 is_lt compares (4x DVE mode). Stats accumulation is
split DVE/ACT/Pool by env knobs; ACT accums use Relu (wz >= 0) to stay
in the loaded activation-table set.
"""

import os
import sys

for _p in ("/opt/trn_rl_repo", "/root/.axon_site/_ro/trn_rl_repo"):
    if _p not in sys.path:
        sys.path.insert(0, _p)

import numpy as np

import concourse.bass as bass
import concourse.bacc as bacc
import concourse.tile as tile
from concourse import mybir
from concourse import bass_utils

F32 = mybir.dt.float32
F16 = mybir.dt.float16
ALU = mybir.AluOpType
ACTF = mybir.ActivationFunctionType

P, T, A = 4096, 128, 32
NCORES = 8
PL = P // NCORES          # 512 population per core
K = int(P * 0.1)          # 409
TEMP, MOM, MIN_STD = 0.5, 0.1, 0.05
INF = 1.0e30

RCH = 8
CHUNK_ROWS = [4, 4] + [RCH] * ((T - 8) // RCH)
NBS = int(os.environ.get("CEM_NBS", "2"))    # f32 staging bufs per pair
NBF = int(os.environ.get("CEM_NBF", "3"))    # f16 arena bufs per pair
W2 = 2 * T + 1            # 257 packed pair width (128 | sep | 128)
NOISE_C0 = int(os.environ.get("CEM_NOISE_C0", "4"))  # first group with a noise slice
NPOOL1 = int(os.environ.get("CEM_NPOOL1", "5"))  # blk1 pool-TT cols
NPOOL2 = int(os.environ.get("CEM_NPOOL2", "4"))  # blk2 pool-TT cols
NACT1 = int(os.environ.get("CEM_NACT1", "7"))    # blk1 ACT-accum cols
NACT2 = int(os.environ.get("CEM_NACT2", "6"))    # blk2 ACT-accum cols
POOLACC = int(os.environ.get("CEM_POOLACC", "0"))  # pool does its own accums
# exact periodic state re-base: subtract SHIFT from the whole DP state after
# each full 8-row group (uniform shift of a min-plus row state is exact; all
# members shift equally so ranks and exp-differences are unchanged). Keeps
# |state| ~ O(2) so the per-row f16 rounding stays ~1e-3 instead of ~2e-2.
SHIFT = float(os.environ.get("CEM_SHIFT", "2.337"))
WIDETT = int(os.environ.get("CEM_WIDETT", "0"))  # one TT spans both chains
GROUP = [list(range(NCORES))]

_CACHE = {}


def _build(stage=9, single=False):
    nc = bacc.Bacc(
        "TRN2",
        target_bir_lowering=False,
        debug=False,
        num_devices=1 if single else NCORES,
    )
    obs_d = nc.dram_tensor("obs", [PL, T, T], F32, kind="ExternalInput")
    means_d = nc.dram_tensor("means", [T, 1, A], F32, kind="ExternalInput")
    stds_d = nc.dram_tensor("stds", [T, 1, A], F32, kind="ExternalInput")
    noise_d = nc.dram_tensor("noise", [T, PL, A], F32, kind="ExternalInput")
    out_d = nc.dram_tensor("out", [2, T, 1, A], F32, kind="ExternalOutput")

    with tile.TileContext(nc) as tc:
        with (
            tc.tile_pool(name="main", bufs=1) as mp,
            tc.tile_pool(name="dram", bufs=1, space="DRAM") as dp,
        ):
            # ---------------- tiles
            means_t = mp.tile([T, A], F32)
            stds_t = mp.tile([T, A], F32)
            bias1_t = mp.tile([T, A], F32)       # means + 1
            two_t = mp.tile([T, 1], F32)
            noise_t = mp.tile([T, PL // 2, A], F32)
            z_t = mp.tile([T, A, PL], F16)       # z = relu(2 - relu(s*n+m+1))
            y_ring = mp.tile([T, 2, PL], F16)

            sar01 = mp.tile([128, NBS, RCH, W2], F32)  # f32 staging, sep=INF
            sar23 = mp.tile([128, NBS, RCH, W2], F32)
            far01 = mp.tile([128, NBF, RCH, W2], F16)  # f16 DP arenas
            far23 = mp.tile([128, NBF, RCH, W2], F16)
            pball = mp.tile([128, 2, W2 + 1], F16)
            uball = mp.tile([128, 2 * W2 + 1], F16)
            pbuf_d = pball[:, 0]
            pbuf_p = pball[:, 1]
            pbflat = pball.rearrange("p a w -> p (a w)")
            ubuf_d = uball[:, 0:W2]
            ubuf_p = uball[:, W2 + 1 : 2 * W2 + 1]

            down16 = mp.tile([128, 4], F16)
            down32 = mp.tile([128, 4], F32)
            gdb = mp.tile([128, 2, P // 2], F16)
            srank = mp.tile([128, P // 2], F16)
            rank8 = mp.tile([128, 2, 4], F32)
            rank4 = mp.tile([128, 4], F32)
            mask4 = mp.tile([128, 4], F32)
            gsb = mp.tile([128, P // 128], F16)
            pmin = mp.tile([128, 1], F16)
            pmb = mp.tile([128, 128], F16)
            ming = mp.tile([128, 1], F16)
            biast = mp.tile([128, 1], F32)
            e4 = mp.tile([128, 4], F32)
            w4 = mp.tile([128, 4], F16)
            epre = mp.tile([128, 1], F32)
            wrow = mp.tile([128, PL], F16)

            wz_d = mp.tile([128, 8, PL], F16)   # DVE-fed ring (2 slabs of 4)
            wzz_d = mp.tile([128, 8, PL], F16)
            wz_p = mp.tile([128, 6, PL], F16)   # Pool-fed ring
            wzz_p = mp.tile([128, 6, PL], F16)
            dump = mp.tile([128, 4, PL], F16)   # ACT-accum dump ring
            dmp2 = mp.tile([128, 4, PL], F16)   # DVE-accum dump ring
            vqu1 = mp.tile([128, 64], F32)      # blk1: a<16: V=2a,Q=2a+1; U=32
            vqu2 = mp.tile([128, 64], F32)      # blk2: a>=16
            rvqu1 = mp.tile([128, 64], F32)
            rvqu2 = mp.tile([128, 64], F32)

            ld16 = dp.tile([PL], F16)
            wl16 = dp.tile([PL], F16)
            gd16 = dp.tile([P], F16)
            pd16 = dp.tile([128], F16)
            arin1 = dp.tile([128 * 64], F32)
            arout1 = dp.tile([128 * 64], F32)
            arin2 = dp.tile([128 * 64], F32)
            arout2 = dp.tile([128 * 64], F32)

            # ---------------- init (before chunk DMAs: arena dep direction)
            nc.vector.memset(sar01[:, :, :, T : T + 1], INF)
            nc.gpsimd.memset(sar23[:, :, :, T : T + 1], INF)
            nc.vector.memset(pbuf_d[:], INF)
            nc.vector.memset(pbuf_d[:, 0:1], 0.0)
            nc.vector.memset(pbuf_d[:, T + 1 : T + 2], 0.0)
            nc.vector.memset(pbuf_p[:], INF)
            nc.vector.memset(pbuf_p[:, 0:1], 0.0)
            nc.vector.memset(pbuf_p[:, T + 1 : T + 2], 0.0)
            nc.vector.memset(two_t[:], 2.0)
            nc.gpsimd.memset(vqu1[:], 0.0)
            nc.gpsimd.memset(vqu2[:], 0.0)

            # scalar-queue DMAs: means/stds then noise slices
            nc.scalar.dma_start(means_t[:], means_d[:, 0, :])
            nc.scalar.dma_start(stds_t[:], stds_d[:, 0, :])
            nc.vector.tensor_scalar(bias1_t[:], means_t[:], 1.0, None, op0=ALU.add)
            NSL = 8
            PSL = PL // NSL

            PH = PL // 2

            def emit_z(h):
                # z = relu(2 - relu(s*n + m + 1)) for population half h
                for a in range(A):
                    nc.scalar.activation(
                        y_ring[:, a % 2, 0:PH],
                        noise_t[:, :, a],
                        ACTF.Relu,
                        bias=bias1_t[:, a : a + 1],
                        scale=stds_t[:, a : a + 1],
                    )
                    nc.scalar.activation(
                        z_t[:, a, h * PH : (h + 1) * PH],
                        y_ring[:, a % 2, 0:PH],
                        ACTF.Relu,
                        bias=two_t[:, 0:1],
                        scale=-1.0,
                    )

            # ---------------- DP: per-group [obs DMAs; pool f32->f16; rows]
            row0 = [0]
            for c, rows in enumerate(CHUNK_ROWS):
                row0.append(row0[-1] + rows)
            NCH = len(CHUNK_ROWS)

            for g in range(NCH):
                rows, r0 = CHUNK_ROWS[g], row0[g]
                bs = g % NBS
                bf = g % NBF
                nc.sync.dma_start(
                    sar01[:, bs, 0:rows, 0:T], obs_d[0:128, r0 : r0 + rows, :]
                )
                nc.sync.dma_start(
                    sar01[:, bs, 0:rows, T + 1 : W2],
                    obs_d[128:256, r0 : r0 + rows, :],
                )
                nc.sync.dma_start(
                    sar23[:, bs, 0:rows, 0:T], obs_d[256:384, r0 : r0 + rows, :]
                )
                nc.sync.dma_start(
                    sar23[:, bs, 0:rows, T + 1 : W2],
                    obs_d[384:512, r0 : r0 + rows, :],
                )
                if NOISE_C0 <= g < NOISE_C0 + 8 and (g - NOISE_C0) % 2 == 0:
                    s = (g - NOISE_C0) // 2
                    nc.sync.dma_start(
                        noise_t[:, s * PSL : (s + 1) * PSL, :],
                        noise_d[:, s * PSL : (s + 1) * PSL, :],
                    )
                if g == NOISE_C0 + 7:
                    emit_z(0)
                # f32 -> f16 (sep col converts to inf); one op per pair
                nc.gpsimd.tensor_copy(
                    far01[:, bf, 0:rows, :], sar01[:, bs, 0:rows, :]
                )
                nc.gpsimd.tensor_copy(
                    far23[:, bf, 0:rows, :], sar23[:, bs, 0:rows, :]
                )
                for rr in range(rows):
                    r = r0 + rr
                    if WIDETT:
                        nc.vector.tensor_tensor(
                            uball[:], pbflat[:, 0 : 2 * W2 + 1],
                            pbflat[:, 1 : 2 * W2 + 2], op=ALU.min,
                        )
                    else:
                        nc.vector.tensor_tensor(
                            ubuf_d, pbuf_d[:, 0:W2], pbuf_d[:, 1 : W2 + 1],
                            op=ALU.min,
                        )
                        nc.vector.tensor_tensor(
                            ubuf_p, pbuf_p[:, 0:W2], pbuf_p[:, 1 : W2 + 1],
                            op=ALU.min,
                        )
                    nc.vector.tensor_tensor_scan(
                        pbuf_d[:, 1 : W2 + 1], ubuf_d, far01[:, bf, rr, :],
                        INF, op0=ALU.min, op1=ALU.add,
                    )
                    nc.vector.tensor_tensor_scan(
                        pbuf_p[:, 1 : W2 + 1], ubuf_p, far23[:, bf, rr, :],
                        INF, op0=ALU.min, op1=ALU.add,
                    )
                    if r == 0:
                        nc.vector.memset(pbuf_d[:, 0:1], INF)
                        nc.vector.memset(pbuf_p[:, 0:1], INF)
                # re-base the DP state after every 8th row except the end
                # (uniform min-plus shift: exact, keeps the f16 state small)
                if (r0 + rows) % RCH == 0 and r0 + rows < T:
                    nc.vector.tensor_scalar(
                        pbflat[:], pbflat[:], -SHIFT, None, op0=ALU.add
                    )

            # ---------------- noise half 1 (ACT queue: after z(0) reads)
            if stage >= 1:
                for s_ in range(4, NSL):
                    nc.scalar.dma_start(
                        noise_t[:, (s_ % 4) * PSL : (s_ % 4 + 1) * PSL, :],
                        noise_d[:, s_ * PSL : (s_ + 1) * PSL, :],
                    )
                emit_z(1)
                # preload Exp+Sqrt tables while the DP finishes
                nc.scalar.activation(epre[:], two_t[:], ACTF.Exp, bias=0.0, scale=1.0)
                nc.scalar.sqrt(epre[:], two_t[:])

            # ---------------- dists -> allgather -> broadcast -> ranks
            nc.vector.tensor_copy(down16[:, 0:1], pbuf_d[:, T : T + 1])
            nc.vector.tensor_copy(down16[:, 1:2], pbuf_d[:, W2 : W2 + 1])
            nc.vector.tensor_copy(down16[:, 2:3], pbuf_p[:, T : T + 1])
            nc.vector.tensor_copy(down16[:, 3:4], pbuf_p[:, W2 : W2 + 1])
            nc.vector.tensor_copy(down32[:], down16[:])

            nc.sync.dma_start(ld16.rearrange("(k p) -> p k", p=128), down16[:])
            if single:
                # split the local AllGather stand-in so the first gdb
                # broadcast can start one hop earlier
                for hh in range(2):
                    _, gsrc4 = bass.broadcast_tensor_aps(
                        gd16[hh * P // 2 : (hh + 1) * P // 2].rearrange(
                            "(r f) -> r f", r=NCORES // 2
                        ),
                        ld16.rearrange("(o f) -> o f", o=1),
                    )
                    nc.sync.dma_start(
                        gd16[hh * P // 2 : (hh + 1) * P // 2].rearrange(
                            "(r f) -> r f", r=NCORES // 2
                        ),
                        gsrc4,
                    )
            else:
                nc.gpsimd.collective_compute(
                    "AllGather",
                    ALU.bypass,
                    replica_groups=GROUP,
                    ins=[ld16.opt()],
                    outs=[gd16.opt()],
                )

            if stage >= 3:
                PH2 = P // 2
                # global-min chain first in the queue (tiny, unblocks pmin so
                # the pd/pmb round-trip hides under the rank compares)
                for h in range(2):
                    _, gsrc = bass.broadcast_tensor_aps(
                        gdb[:, h],
                        gd16[h * PH2 : (h + 1) * PH2].rearrange(
                            "(o f) -> o f", o=1
                        ),
                    )
                    nc.sync.dma_start(gdb[:, h], gsrc)
                nc.sync.dma_start(
                    gsb[:], gd16.rearrange("(p q) -> p q", p=128)
                )
                nc.vector.tensor_reduce(
                    pmin[:], gsb[:], axis=mybir.AxisListType.X, op=ALU.min
                )
                nc.sync.dma_start(
                    pd16.rearrange("(p q) -> p q", q=1), pmin[:]
                )
                _, psrc = bass.broadcast_tensor_aps(
                    pmb[:], pd16.rearrange("(o f) -> o f", o=1)
                )
                nc.sync.dma_start(pmb[:], psrc)

                for h in range(2):
                    for k in range(4):
                        nc.vector.tensor_scalar(
                            srank[:],
                            gdb[:, h],
                            down32[:, k : k + 1],
                            None,
                            op0=ALU.is_lt,
                            op1=ALU.add,
                            accum_out=rank8[:, h, k : k + 1],
                        )
                nc.vector.tensor_reduce(
                    ming[:], pmb[:], axis=mybir.AxisListType.X, op=ALU.min
                )
                nc.vector.tensor_scalar(
                    biast[:], ming[:], TEMP, None, op0=ALU.mult
                )
                nc.vector.tensor_tensor(
                    rank4[:], rank8[:, 0], rank8[:, 1], op=ALU.add
                )
                nc.vector.tensor_scalar(
                    mask4[:], rank4[:], float(K), None, op0=ALU.is_lt
                )
                nc.scalar.activation(
                    e4[:], down16[:], ACTF.Exp, bias=biast[:, 0:1], scale=-TEMP
                )
                nc.vector.tensor_tensor(w4[:], e4[:], mask4[:], op=ALU.mult)

                nc.scalar.dma_start(wl16.rearrange("(k p) -> p k", p=128), w4[:])
                _, wsrc = bass.broadcast_tensor_aps(
                    wrow[:], wl16.rearrange("(o f) -> o f", o=1)
                )
                nc.scalar.dma_start(wrow[:], wsrc)

            if stage >= 5:
                _actc = [0]
                _dvec = [0]

                def adump():
                    _actc[0] += 1
                    return dump[:, _actc[0] % 4]

                def ddump():
                    _dvec[0] += 1
                    return dmp2[:, _dvec[0] % 4]

                U = vqu1[:, 32:33]
                nc.vector.tensor_scalar(
                    ddump(), wrow[:], 0.0, None, op0=ALU.add, op1=ALU.add,
                    accum_out=U,
                )

                def V(a):
                    blk = vqu1 if a < 16 else vqu2
                    return blk[:, 2 * (a % 16) : 2 * (a % 16) + 1]

                def Q(a):
                    blk = vqu1 if a < 16 else vqu2
                    return blk[:, 2 * (a % 16) + 1 : 2 * (a % 16) + 2]

                # per 16-col block: pool-TT cols (DVE accums), DVE-TT cols
                # with ACT accums, then DVE-full cols. Block 1 first so its
                # AllReduce overlaps block-2 stats.
                for blk in range(2):
                    a0 = blk * 16
                    npool_b = NPOOL1 if blk == 0 else NPOOL2
                    nact_b = NACT1 if blk == 0 else NACT2
                    cols_pool = list(range(a0, a0 + npool_b))
                    cols_act = list(range(a0 + npool_b, a0 + npool_b + nact_b))
                    cols_dve = list(range(a0 + npool_b + nact_b, a0 + 16))
                    # pool TTs issue first (they gate nothing on DVE)
                    for i, a in enumerate(cols_pool):
                        za = z_t[:, a, :]
                        s = (blk * NPOOL1 + i) % 6
                        nc.gpsimd.tensor_tensor(
                            wz_p[:, s], za, wrow[:], op=ALU.mult
                        )
                        nc.gpsimd.tensor_tensor(
                            wzz_p[:, s], wz_p[:, s], za, op=ALU.mult
                        )
                    # DVE work: ACT-accum cols first, then DVE-full cols,
                    # then the pool-col accums (pool results ready by then)
                    for i, a in enumerate(cols_act):
                        za = z_t[:, a, :]
                        wz, wzz = wz_d[:, a % 4], wzz_d[:, a % 4]
                        nc.vector.tensor_tensor(wz, za, wrow[:], op=ALU.mult)
                        nc.vector.tensor_tensor(wzz, wz, za, op=ALU.mult)
                        nc.scalar.activation(
                            adump(), wz, ACTF.Relu, accum_out=V(a)
                        )
                        nc.scalar.activation(
                            adump(), wzz, ACTF.Relu, accum_out=Q(a)
                        )
                    for a in cols_dve:
                        za = z_t[:, a, :]
                        wz = wz_d[:, a % 4]
                        nc.vector.tensor_tensor(wz, za, wrow[:], op=ALU.mult)
                        nc.vector.tensor_scalar(
                            ddump(), wz, 0.0, None, op0=ALU.add,
                            op1=ALU.add, accum_out=V(a),
                        )
                        nc.vector.tensor_tensor(wz, wz, za, op=ALU.mult)
                        nc.vector.tensor_scalar(
                            ddump(), wz, 0.0, None, op0=ALU.add,
                            op1=ALU.add, accum_out=Q(a),
                        )
                    for i, a in enumerate(cols_pool):
                        s = (blk * NPOOL1 + i) % 6
                        nc.vector.tensor_scalar(
                            ddump(), wz_p[:, s], 0.0, None, op0=ALU.add,
                            op1=ALU.add, accum_out=V(a),
                        )
                        nc.vector.tensor_scalar(
                            ddump(), wzz_p[:, s], 0.0, None, op0=ALU.add,
                            op1=ALU.add, accum_out=Q(a),
                        )
                    vin = vqu1 if blk == 0 else vqu2
                    ain = arin1 if blk == 0 else arin2
                    aout = arout1 if blk == 0 else arout2
                    rv = rvqu1 if blk == 0 else rvqu2
                    q_ar = nc.sync if blk == 0 else nc.scalar
                    q_ar.dma_start(
                        ain.rearrange("(p a) -> p a", a=64), vin[:]
                    )
                    if single:
                        q_ar.dma_start(aout[:], ain[:])
                    else:
                        nc.gpsimd.collective_compute(
                            "AllReduce", ALU.add, replica_groups=GROUP,
                            ins=[ain.opt()], outs=[aout.opt()],
                        )
                    q_ar.dma_start(
                        rv[:], aout.rearrange("(p a) -> p a", a=64)
                    )

                # ---------------- final statistics (block 1 math first)
                inv = mp.tile([128, 1], F32)
                mh = mp.tile([128, A], F32)
                mstd = mp.tile([128, 2, A], F32)
                q = mp.tile([128, A], F32)
                t2 = mp.tile([128, A], F32)
                msq = mp.tile([128, A], F32)
                var = mp.tile([128, A], F32)
                rU = rvqu1[:, 32:33]
                nc.vector.reciprocal(inv[:], rU)
                for blk in range(2):
                    rsrc = rvqu1 if blk == 0 else rvqu2
                    n0 = blk * 16
                    rV = rsrc[:, 0:32:2]
                    rQ = rsrc[:, 1:32:2]
                    mhB = mh[:, n0 : n0 + 16]
                    qB = q[:, n0 : n0 + 16]
                    t2B = t2[:, n0 : n0 + 16]
                    msqB = msq[:, n0 : n0 + 16]
                    varB = var[:, n0 : n0 + 16]
                    stdB = mstd[:, 1, n0 : n0 + 16]
                    # num1 = U - V ; mean_hat = num1 / U
                    nc.vector.tensor_scalar(
                        t2B, rV, rU[:, 0:1], -1.0, op0=ALU.subtract, op1=ALU.mult
                    )
                    nc.vector.tensor_scalar(mhB, t2B, inv[:, 0:1], None, op0=ALU.mult)
                    # num2 = U - 2V + Q ; q = num2 / U
                    nc.vector.tensor_scalar(t2B, rV, -2.0, None, op0=ALU.mult)
                    nc.vector.tensor_tensor(t2B, t2B, rQ, op=ALU.add)
                    nc.vector.tensor_scalar(t2B, t2B, rU[:, 0:1], None, op0=ALU.add)
                    nc.vector.tensor_scalar(qB, t2B, inv[:, 0:1], None, op0=ALU.mult)
                    nc.vector.tensor_tensor(msqB, mhB, mhB, op=ALU.mult)
                    nc.vector.tensor_tensor(varB, qB, msqB, op=ALU.subtract)
                    nc.vector.tensor_scalar(varB, varB, 0.0, None, op0=ALU.max)
                    nc.scalar.sqrt(stdB, varB)
                    nc.vector.tensor_scalar(
                        stdB, stdB, MIN_STD, 1.0, op0=ALU.max, op1=ALU.min
                    )
                    nc.vector.tensor_scalar(mhB, mhB, 1.0 - MOM, None, op0=ALU.mult)
                    nc.vector.scalar_tensor_tensor(
                        mstd[:, 0, n0 : n0 + 16], means_t[:, n0 : n0 + 16], MOM,
                        mhB, op0=ALU.mult, op1=ALU.add,
                    )
                nc.sync.dma_start(out_d[0, :, 0, :], mstd[:, 0, :])
                nc.scalar.dma_start(out_d[1, :, 0, :], mstd[:, 1, :])
            else:
                dbg = mp.tile([128, A], F32)
                nc.vector.memset(dbg[:], 0.0)
                if stage >= 3:
                    nc.vector.tensor_copy(dbg[:, 0:4], w4[:])
                    nc.vector.tensor_copy(dbg[:, 4:8], rank4[:])
                else:
                    nc.vector.tensor_copy(dbg[:, 0:4], down16[:])
                nc.scalar.dma_start(out_d[0, :, 0, :], dbg[:])
                nc.scalar.dma_start(out_d[1, :, 0, :], dbg[:])

    nc.compile()
    return nc


def _get_nc(stage=None, single=None):
    if stage is None:
        stage = int(os.environ.get("CEM_STAGE", "9"))
    if single is None:
        single = bool(int(os.environ.get("CEM_SINGLE", "0")))
    key = ("nc", stage, single)
    if key not in _CACHE:
        _CACHE[key] = _build(stage, single)
    return _CACHE[key]


def kernel(**inputs):
    obs = np.ascontiguousarray(np.asarray(inputs["obs_diffs"], np.float32))
    means = np.ascontiguousarray(np.asarray(inputs["means"], np.float32))
    stds = np.ascontiguousarray(np.asarray(inputs["stds"], np.float32))
    noise = np.ascontiguousarray(np.asarray(inputs["noise"], np.float32))

    nc = _get_nc(stage=9, single=False)
    in_maps = []
    for c in range(NCORES):
        in_maps.append(
            {
                "obs": obs[c * PL : (c + 1) * PL],
                "means": means,
                "stds": stds,
                "noise": np.ascontiguousarray(noise[:, c * PL : (c + 1) * PL, :]),
            }
        )
    res = bass_utils.run_bass_kernel_spmd(
        nc, in_maps, core_ids=list(range(NCORES))
    )
    out = np.asarray(res.results[0]["out"], np.float32)
    return out.reshape(2, T, 1, A)
